# revision 25
# baseline (speedup 1.0000x reference)
"""Bass/Trainium2 kernel for nn_Attention (Bahdanau-style additive attention).

reference:
    inp = input @ W_in.T + b_in                                  # [B, H]
    ctx = einsum('bsd,hd->bhs', context, W_ctx) + b_ctx          # [B, H, S]
    att = einsum('h,bhs->bs', V, tanh(inp[:,:,None] + ctx))      # [B, S]
    att = where(mask, -inf, att); alpha = softmax(att, -1)       # [B, S]
    hidden = einsum('bhs,bs->bh', ctx, alpha)                    # [B, H]

Key restructuring: hidden = W_ctx @ (alpha @ context) + b_ctx (since sum(alpha)=1),
so the big [B,H,S] ctx tensor is only ever materialized tile-by-tile in PSUM.

Sharding: data-parallel over batch B across 8 cores (16 batches/core).
Compute dtype fp16 (alpha abs err ~2e-3 vs f32 reference), f32 PSUM accumulate.
Batches processed in 2 groups of 8 so group-0 softmax/pass-B overlaps group-1
score compute.
"""
import os
import numpy as np

B, S, D, H = 128, 1024, 512, 512
N_CORES = 8
BS = B // N_CORES   # batches per core
NG = 2              # batch groups per core
GS = BS // NG       # batches per group
KT = D // 128       # 4 contraction tiles
MT = H // 128       # 4 output tiles
ST = S // 128       # 8 sequence tiles
NEG = -1.0e30

_cache = {"nc": None}


def _build_nc():
    import concourse.bacc as bacc
    import concourse.tile as tile
    from concourse import mybir

    fp16 = mybir.dt.float16
    f32 = mybir.dt.float32
    TANH = mybir.ActivationFunctionType.Tanh
    EXP = mybir.ActivationFunctionType.Exp
    X = mybir.AxisListType.X
    MAX = mybir.AluOpType.max
    ADD = mybir.AluOpType.add

    nc = bacc.Bacc("TRN2", target_bir_lowering=False, debug=False,
                   num_devices=N_CORES)

    ctx16 = nc.declare_dram_parameter("ctx16", [BS * S, D], fp16, isOutput=False)
    ctxT16 = nc.declare_dram_parameter("ctxT16", [BS * D, S], fp16, isOutput=False)
    WCA = H + 1              # [W_ctxT | V]
    WCB = H + BS             # [W_inT | inputT]
    WpackA = nc.declare_dram_parameter("WpackA", [D, WCA], fp16, isOutput=False)
    WpackB = nc.declare_dram_parameter("WpackB", [D, WCB], fp16, isOutput=False)
    rows16 = nc.declare_dram_parameter("rows16", [1, 2 * H + BS], fp16, isOutput=False)
    maskadd = nc.declare_dram_parameter("maskadd", [BS, S], f32, isOutput=False)
    ident16 = nc.declare_dram_parameter("ident16", [GS, GS], fp16, isOutput=False)

    alpha_out = nc.declare_dram_parameter("alpha_out", [BS, S], f32, isOutput=True)
    hidT_out = nc.declare_dram_parameter("hidT_out", [H, BS], f32, isOutput=True)

    with tile.TileContext(nc) as tc:
        with tc.tile_pool(name="const", bufs=1) as cpool, \
             tc.tile_pool(name="natc", bufs=1) as natpool, \
             tc.tile_pool(name="dramp", bufs=1, space="DRAM") as dpool:

            # weights / small constants (live whole kernel)
            wpa = cpool.tile([128, KT * WCA], fp16, tag="wpa", name="wpa")
            nc.gpsimd.dma_start(
                wpa[:].rearrange("p (k c) -> p k c", k=KT),
                WpackA[:, :].rearrange("(k p) c -> p k c", k=KT))
            wctx = [wpa[:, k * WCA:k * WCA + H] for k in range(KT)]
            vcol = [wpa[:, k * WCA + H:k * WCA + WCA] for k in range(KT)]
            rp = cpool.tile([1, 2 * H + BS], fp16, tag="rp", name="rp")
            wpb = cpool.tile([128, KT * WCB], fp16, tag="wpb", name="wpb")
            winT = [wpb[:, k * WCB:k * WCB + H] for k in range(KT)]
            inT = [wpb[:, k * WCB + H:k * WCB + WCB] for k in range(KT)]
            brow_sb = rp[:, 0:H]
            bctx_sb = rp[:, H:2 * H]
            ones_sb = rp[:, 2 * H:2 * H + BS]
            ident_sb = cpool.tile([GS, GS], fp16, tag="ident", name="ident")
            nc.gpsimd.dma_start(ident_sb[:], ident16[:, :])
            # per-(h,b) tanh bias: W_in @ input.T + (b_in + b_ctx), f32, [128, BS] x MT
            ibias = [cpool.tile([128, BS], f32, tag=f"ibias{m}", name=f"ibias{m}") for m in range(MT)]

            nat = [None] * BS  # natural-layout ctx tiles for pass B
            att_gs = [None] * NG
            c_alls = [None] * NG

            with tc.tile_pool(name="ctxT", bufs=3) as tpool, \
                 tc.tile_pool(name="tanh", bufs=2) as hpool, \
                 tc.tile_pool(name="mmps", bufs=6, space="PSUM") as mmps, \
                 tc.tile_pool(name="attps", bufs=2, space="PSUM") as attps, \
                 tc.tile_pool(name="smx", bufs=1) as smx, \
                 tc.tile_pool(name="p2w", bufs=2) as p2w:

                def phase0():
                    # load rows + W_inT|inputT; ibias = W_in@input.T + biases
                    nc.sync.dma_start(rp[:], rows16[:, :])
                    nc.sync.dma_start(
                        wpb[:].rearrange("p (k c) -> p k c", k=KT),
                        WpackB[:, :].rearrange("(k p) c -> p k c", k=KT))
                    for m in range(MT):
                        ps = attps.tile([128, BS], f32, tag="att", name="p0ps")
                        for k in range(KT):
                            nc.tensor.matmul(ps[:], winT[k][:, m * 128:(m + 1) * 128],
                                             inT[k][:], start=(k == 0), stop=False)
                        nc.tensor.matmul(ps[:], brow_sb[:, m * 128:(m + 1) * 128],
                                         ones_sb[:], start=False, stop=True)
                        nc.scalar.copy(ibias[m][:], ps[:])

                def scores_group(g):
                    """Phase 1 for batches g*GS .. (g+1)*GS-1: att rows -> att_g."""
                    att_g = smx.tile([GS, S], f32, tag="att_g", name="att_g", bufs=2)
                    att_gs[g] = att_g
                    for j in range(GS):
                        b = g * GS + j
                        row0 = b * S
                        trow0 = b * D
                        # pre-transposed context, one 1MB DMA: [128, (k s)]
                        ctxTb = tpool.tile([128, KT * S], fp16, tag="ctxTb",
                                           name="ctxTb")
                        nc.sync.dma_start(
                            ctxTb[:].rearrange("p (k s) -> p k s", k=KT),
                            ctxT16[trow0:trow0 + D, :].rearrange(
                                "(k p) s -> p k s", k=KT))
                        tanh_sb = [hpool.tile([128, S], fp16, tag=f"tanh{m}",
                                              name=f"tanh{m}") for m in range(MT)]
                        chunk_ps = {}
                        for n in range(2):
                            for m in range(MT):
                                ps = mmps.tile([128, 512], f32, tag="mm", name="mm")
                                for k in range(KT):
                                    nc.tensor.matmul(
                                        ps[:], wctx[k][:, m * 128:(m + 1) * 128],
                                        ctxTb[:, k * S + n * 512:k * S + (n + 1) * 512],
                                        start=(k == 0), stop=(k == KT - 1))
                                if b == 0 and n == 0:
                                    chunk_ps[m] = ps
                                    continue
                                nc.scalar.activation(
                                    tanh_sb[m][:, n * 512:(n + 1) * 512], ps[:],
                                    TANH, bias=ibias[m][:, b:b + 1])
                            if b == 0 and n == 0:
                                phase0()
                                for m in range(MT):
                                    nc.scalar.activation(
                                        tanh_sb[m][:, 0:512], chunk_ps[m][:],
                                        TANH, bias=ibias[m][:, b:b + 1])
                        for n in range(2):
                            ap = attps.tile([1, 512], f32, tag="att", name="att")
                            for m in range(MT):
                                nc.tensor.matmul(ap[:], vcol[m][:, 0:1],
                                                 tanh_sb[m][:, n * 512:(n + 1) * 512],
                                                 start=(m == 0), stop=(m == MT - 1))
                            arow = hpool.tile([1, 512], f32, tag="arow", name="arow")
                            nc.vector.tensor_copy(arow[:], ap[0:1, :])
                            nc.gpsimd.dma_start(
                                att_g[j:j + 1, n * 512:(n + 1) * 512], arow[:])
                        # natural load for pass B (SWDGE), one 1MB DMA - after
                        # compute so the score-path ctxT DMAs get HBM priority
                        natb = natpool.tile([128, ST * D], fp16, tag=f"natb{b}",
                                            name=f"natb{b}")
                        nc.vector.memset(natb[0:1, 0:1], 0)
                        nc.gpsimd.dma_start(
                            natb[:].rearrange("p (t d) -> p t d", t=ST),
                            ctx16[row0:row0 + S, :].rearrange(
                                "(t p) d -> p t d", t=ST))
                        nat[b] = natb

                alpha16s = [None] * NG
                alphaTs = [None] * NG

                def smx_group(g):
                    """softmax for group g."""
                    r0 = g * GS
                    att_g = att_gs[g]
                    mask_g = smx.tile([GS, S], f32, tag="mask_g", name="mask_g")
                    nc.gpsimd.dma_start(mask_g[:], maskadd[r0:r0 + GS, :])
                    nc.vector.tensor_add(att_g[:], att_g[:], mask_g[:])
                    negmx = smx.tile([GS, 1], f32, tag="negmx", name="negmx")
                    nc.vector.tensor_reduce(negmx[:], att_g[:], axis=X, op=MAX, negate=True)
                    p16 = smx.tile([GS, S], fp16, tag="p16", name="p16")
                    nc.scalar.activation(p16[:], att_g[:], EXP, bias=negmx[:])
                    ssum = smx.tile([GS, 1], f32, tag="ssum", name="ssum")
                    nc.vector.tensor_reduce(ssum[:], p16[:], axis=X, op=ADD)
                    rsum = smx.tile([GS, 1], f32, tag="rsum", name="rsum")
                    nc.vector.reciprocal(rsum[:], ssum[:])
                    nc.vector.tensor_scalar_mul(att_g[:], p16[:], rsum[:])
                    nc.gpsimd.dma_start(alpha_out[r0:r0 + GS, :], att_g[:])
                    alpha16 = smx.tile([GS, S], fp16, tag="alpha16", name="alpha16",
                                       bufs=2)
                    nc.vector.tensor_scalar_mul(alpha16[:], p16[:], rsum[:])
                    alpha16s[g] = alpha16

                def passB_group(g):
                    """alphaT + weighted context sum for group g."""
                    r0 = g * GS
                    alpha16 = alpha16s[g]
                    c_all = p2w.tile([GS, D], fp16, tag="c_all", name="c_all")
                    c_alls[g] = c_all
                    # alphaT[st] [128, GS] via PE transpose
                    alphaT = [p2w.tile([128, GS], fp16, tag=f"alT{st}",
                                        name=f"alT{st}") for st in range(ST)]
                    alphaTs[g] = alphaT
                    for st in range(ST):
                        pt = mmps.tile([128, GS], fp16, tag="mm", name="pt16")
                        nc.tensor.transpose(pt[:], alpha16[:, st * 128:(st + 1) * 128],
                                            ident_sb[:, :])
                        nc.scalar.copy(alphaT[st][:], pt[:])
                    # c[b] = alpha[b] @ context[b]  (rows via DRAM bounce)
                    for j in range(GS):
                        b = r0 + j
                        pc = attps.tile([1, D], f32, tag="att", name="pc")
                        for st in range(ST):
                            nc.tensor.matmul(pc[:], alphaT[st][:, j:j + 1],
                                             nat[b][:, st * D:(st + 1) * D],
                                             start=(st == 0), stop=(st == ST - 1))
                        crow = p2w.tile([1, D], fp16, tag="crow", name="crow")
                        nc.vector.tensor_copy(crow[:], pc[0:1, :])
                        nc.gpsimd.dma_start(c_all[j:j + 1, :], crow[:])

                def proj_group(g):
                    """cT + output projection for group g."""
                    r0 = g * GS
                    c_all = c_alls[g]
                    cT = [p2w.tile([128, GS], fp16, tag=f"cT{k}", name=f"cT{k}")
                          for k in range(KT)]
                    for k in range(KT):
                        ptc = mmps.tile([128, GS], fp16, tag="mm", name="ptc")
                        nc.tensor.transpose(ptc[:], c_all[:, k * 128:(k + 1) * 128],
                                            ident_sb[:, :])
                        nc.scalar.copy(cT[k][:], ptc[:])
                    # hidden.T[:, group] = W_ctx @ c + b_ctx
                    hT = p2w.tile([128, MT * GS], f32, tag="hT", name="hT")
                    for m in range(MT):
                        ph = mmps.tile([128, GS], f32, tag="mm", name="ph")
                        for k in range(KT):
                            nc.tensor.matmul(ph[:], wctx[k][:, m * 128:(m + 1) * 128],
                                             cT[k][:], start=(k == 0), stop=False)
                        nc.tensor.matmul(ph[:], bctx_sb[:, m * 128:(m + 1) * 128],
                                         ones_sb[:, 0:GS], start=False, stop=True)
                        nc.scalar.copy(hT[:, m * GS:(m + 1) * GS], ph[:])
                    nc.gpsimd.dma_start(
                        hidT_out[:, r0:r0 + GS].rearrange("(m p) b -> p m b", m=MT),
                        hT[:].rearrange("p (m b) -> p m b", m=MT))

                for g in range(NG):
                    scores_group(g)
                for g in range(NG):
                    smx_group(g)
                for g in range(NG):
                    passB_group(g)
                for g in range(NG):
                    proj_group(g)

    nc.finalize()
    return nc


def _prep_core_inputs(inputs):
    """Host-side sharding + layout prep. Returns in_maps for 8 cores."""
    context = np.ascontiguousarray(inputs["context"], dtype=np.float32)
    inp = np.asarray(inputs["input"], dtype=np.float32)
    mask = np.asarray(inputs["mask"])
    W_in = np.asarray(inputs["W_in"], dtype=np.float32)
    b_in = np.asarray(inputs["b_in"], dtype=np.float32)
    W_ctx = np.asarray(inputs["W_ctx"], dtype=np.float32)
    b_ctx = np.asarray(inputs["b_ctx"], dtype=np.float32)
    V = np.asarray(inputs["V"], dtype=np.float32)

    ident16 = np.eye(GS, dtype=np.float16)
    rows16 = np.concatenate([
        (b_in + b_ctx).astype(np.float16), b_ctx.astype(np.float16),
        np.ones(BS, np.float16)]).reshape(1, 2 * H + BS)

    ctx16_c = []
    ctxT16_c = []
    for c in range(N_CORES):
        bsl = slice(c * BS, (c + 1) * BS)
        blk16 = context[bsl].astype(np.float16)
        ctx16_c.append(blk16.reshape(BS * S, D))
        ctxT16_c.append(np.ascontiguousarray(blk16.transpose(0, 2, 1)).reshape(BS * D, S))
    WpackA = np.empty((D, H + 1), np.float16)
    WpackA[:, 0:H] = W_ctx.T.astype(np.float16)
    WpackA[:, H] = V.astype(np.float16)
    WinT = W_in.T.astype(np.float16)
    in_maps = []
    for c in range(N_CORES):
        bsl = slice(c * BS, (c + 1) * BS)
        WpackB = np.empty((D, H + BS), np.float16)
        WpackB[:, 0:H] = WinT
        WpackB[:, H:] = inp[bsl].T.astype(np.float16)
        in_maps.append({
            "ctx16": ctx16_c[c],
            "ctxT16": ctxT16_c[c],
            "WpackA": WpackA,
            "WpackB": WpackB,
            "rows16": rows16,
            "maskadd": np.where(mask[bsl], np.float32(NEG), np.float32(0.0)),
            "ident16": ident16,
        })
    return in_maps


def kernel(**inputs):
    from concourse.bass_utils import run_bass_kernel_spmd

    if _cache["nc"] is None:
        _cache["nc"] = _build_nc()
    nc = _cache["nc"]

    in_maps = _prep_core_inputs(inputs)
    kwargs = {}
    trace = bool(os.environ.get("KERNEL_TRACE"))
    if trace:
        try:
            import prof_hook
            prof_hook.install()
        except Exception:
            pass
        kwargs = {"trace": True, "tmpdir": os.environ.get("KERNEL_TRACE_DIR") or None}
    res = run_bass_kernel_spmd(nc, in_maps, core_ids=list(range(N_CORES)), **kwargs)
    _cache["last_exec_ns"] = res.exec_time_ns

    hidden = np.empty((B, H), np.float32)
    alpha = np.empty((B, S), np.float32)
    for c in range(N_CORES):
        bsl = slice(c * BS, (c + 1) * BS)
        alpha[bsl] = res.results[c]["alpha_out"]
        hidden[bsl] = res.results[c]["hidT_out"].T
    return hidden, alpha


# revision 26
# speedup vs baseline: 1.0084x; 1.0084x over previous
"""Bass/Trainium2 kernel for nn_Attention (Bahdanau-style additive attention).

reference:
    inp = input @ W_in.T + b_in                                  # [B, H]
    ctx = einsum('bsd,hd->bhs', context, W_ctx) + b_ctx          # [B, H, S]
    att = einsum('h,bhs->bs', V, tanh(inp[:,:,None] + ctx))      # [B, S]
    att = where(mask, -inf, att); alpha = softmax(att, -1)       # [B, S]
    hidden = einsum('bhs,bs->bh', ctx, alpha)                    # [B, H]

Key restructuring: hidden = W_ctx @ (alpha @ context) + b_ctx (since sum(alpha)=1),
so the big [B,H,S] ctx tensor is only ever materialized tile-by-tile in PSUM.

Sharding: data-parallel over batch B across 8 cores (16 batches/core).
Compute dtype fp16 (alpha abs err ~2e-3 vs f32 reference), f32 PSUM accumulate.
Batches processed in 2 groups of 8 so group-0 softmax/pass-B overlaps group-1
score compute.
"""
import os
import numpy as np

B, S, D, H = 128, 1024, 512, 512
N_CORES = 8
BS = B // N_CORES   # batches per core
NG = 2              # batch groups per core
GS = BS // NG       # batches per group
KT = D // 128       # 4 contraction tiles
MT = H // 128       # 4 output tiles
ST = S // 128       # 8 sequence tiles
NEG = -1.0e30

_cache = {"nc": None}


def _build_nc():
    import concourse.bacc as bacc
    import concourse.tile as tile
    from concourse import mybir

    fp16 = mybir.dt.float16
    f32 = mybir.dt.float32
    TANH = mybir.ActivationFunctionType.Tanh
    EXP = mybir.ActivationFunctionType.Exp
    X = mybir.AxisListType.X
    MAX = mybir.AluOpType.max
    ADD = mybir.AluOpType.add

    nc = bacc.Bacc("TRN2", target_bir_lowering=False, debug=False,
                   num_devices=N_CORES)

    ctx16 = nc.declare_dram_parameter("ctx16", [BS * S, D], fp16, isOutput=False)
    ctxT16 = nc.declare_dram_parameter("ctxT16", [BS * D, S], fp16, isOutput=False)
    WCA = H + 1              # [W_ctxT | V]
    WCB = H + BS             # [W_inT | inputT]
    WpackA = nc.declare_dram_parameter("WpackA", [D, WCA], fp16, isOutput=False)
    WpackB = nc.declare_dram_parameter("WpackB", [D, WCB], fp16, isOutput=False)
    rows16 = nc.declare_dram_parameter("rows16", [1, 2 * H + BS], fp16, isOutput=False)
    maskadd = nc.declare_dram_parameter("maskadd", [BS, S], f32, isOutput=False)
    ident16 = nc.declare_dram_parameter("ident16", [GS, GS], fp16, isOutput=False)

    alpha_out = nc.declare_dram_parameter("alpha_out", [BS, S], f32, isOutput=True)
    hidT_out = nc.declare_dram_parameter("hidT_out", [H, BS], f32, isOutput=True)

    with tile.TileContext(nc) as tc:
        with tc.tile_pool(name="const", bufs=1) as cpool, \
             tc.tile_pool(name="natc", bufs=1) as natpool, \
             tc.tile_pool(name="dramp", bufs=1, space="DRAM") as dpool:

            # weights / small constants (live whole kernel)
            wpa = cpool.tile([128, KT * WCA], fp16, tag="wpa", name="wpa")
            nc.sync.dma_start(
                wpa[:].rearrange("p (k c) -> p k c", k=KT),
                WpackA[:, :].rearrange("(k p) c -> p k c", k=KT))
            wctx = [wpa[:, k * WCA:k * WCA + H] for k in range(KT)]
            vcol = [wpa[:, k * WCA + H:k * WCA + WCA] for k in range(KT)]
            rp = cpool.tile([1, 2 * H + BS], fp16, tag="rp", name="rp")
            wpb = cpool.tile([128, KT * WCB], fp16, tag="wpb", name="wpb")
            winT = [wpb[:, k * WCB:k * WCB + H] for k in range(KT)]
            inT = [wpb[:, k * WCB + H:k * WCB + WCB] for k in range(KT)]
            brow_sb = rp[:, 0:H]
            bctx_sb = rp[:, H:2 * H]
            ones_sb = rp[:, 2 * H:2 * H + BS]
            ident_sb = cpool.tile([GS, GS], fp16, tag="ident", name="ident")
            nc.gpsimd.dma_start(ident_sb[:], ident16[:, :])
            # per-(h,b) tanh bias: W_in @ input.T + (b_in + b_ctx), f32, [128, BS] x MT
            ibias = [cpool.tile([128, BS], f32, tag=f"ibias{m}", name=f"ibias{m}") for m in range(MT)]

            nat = [None] * BS  # natural-layout ctx tiles for pass B
            att_gs = [None] * NG
            c_alls = [None] * NG

            with tc.tile_pool(name="ctxT", bufs=3) as tpool, \
                 tc.tile_pool(name="tanh", bufs=2) as hpool, \
                 tc.tile_pool(name="mmps", bufs=6, space="PSUM") as mmps, \
                 tc.tile_pool(name="attps", bufs=2, space="PSUM") as attps, \
                 tc.tile_pool(name="smx", bufs=1) as smx, \
                 tc.tile_pool(name="p2w", bufs=2) as p2w:

                def phase0():
                    # load rows + W_inT|inputT; ibias = W_in@input.T + biases
                    nc.sync.dma_start(rp[:], rows16[:, :])
                    nc.sync.dma_start(
                        wpb[:].rearrange("p (k c) -> p k c", k=KT),
                        WpackB[:, :].rearrange("(k p) c -> p k c", k=KT))
                    for m in range(MT):
                        ps = attps.tile([128, BS], f32, tag="att", name="p0ps")
                        for k in range(KT):
                            nc.tensor.matmul(ps[:], winT[k][:, m * 128:(m + 1) * 128],
                                             inT[k][:], start=(k == 0), stop=False)
                        nc.tensor.matmul(ps[:], brow_sb[:, m * 128:(m + 1) * 128],
                                         ones_sb[:], start=False, stop=True)
                        nc.scalar.copy(ibias[m][:], ps[:])

                def scores_group(g):
                    """Phase 1 for batches g*GS .. (g+1)*GS-1: att rows -> att_g."""
                    att_g = smx.tile([GS, S], f32, tag="att_g", name="att_g", bufs=2)
                    att_gs[g] = att_g
                    for j in range(GS):
                        b = g * GS + j
                        row0 = b * S
                        trow0 = b * D
                        # pre-transposed context, one 1MB DMA: [128, (k s)]
                        ctxTb = tpool.tile([128, KT * S], fp16, tag="ctxTb",
                                           name="ctxTb")
                        nc.sync.dma_start(
                            ctxTb[:].rearrange("p (k s) -> p k s", k=KT),
                            ctxT16[trow0:trow0 + D, :].rearrange(
                                "(k p) s -> p k s", k=KT))
                        tanh_sb = [hpool.tile([128, S], fp16, tag=f"tanh{m}",
                                              name=f"tanh{m}") for m in range(MT)]
                        chunk_ps = {}
                        for n in range(2):
                            for m in range(MT):
                                ps = mmps.tile([128, 512], f32, tag="mm", name="mm")
                                for k in range(KT):
                                    nc.tensor.matmul(
                                        ps[:], wctx[k][:, m * 128:(m + 1) * 128],
                                        ctxTb[:, k * S + n * 512:k * S + (n + 1) * 512],
                                        start=(k == 0), stop=(k == KT - 1))
                                if b == 0 and n == 0:
                                    chunk_ps[m] = ps
                                    continue
                                nc.scalar.activation(
                                    tanh_sb[m][:, n * 512:(n + 1) * 512], ps[:],
                                    TANH, bias=ibias[m][:, b:b + 1])
                            if b == 0 and n == 0:
                                phase0()
                                for m in range(MT):
                                    nc.scalar.activation(
                                        tanh_sb[m][:, 0:512], chunk_ps[m][:],
                                        TANH, bias=ibias[m][:, b:b + 1])
                        for n in range(2):
                            ap = attps.tile([1, 512], f32, tag="att", name="att")
                            for m in range(MT):
                                nc.tensor.matmul(ap[:], vcol[m][:, 0:1],
                                                 tanh_sb[m][:, n * 512:(n + 1) * 512],
                                                 start=(m == 0), stop=(m == MT - 1))
                            arow = hpool.tile([1, 512], f32, tag="arow", name="arow")
                            nc.vector.tensor_copy(arow[:], ap[0:1, :])
                            nc.gpsimd.dma_start(
                                att_g[j:j + 1, n * 512:(n + 1) * 512], arow[:])
                        # natural load for pass B (SWDGE), one 1MB DMA - after
                        # compute so the score-path ctxT DMAs get HBM priority
                        natb = natpool.tile([128, ST * D], fp16, tag=f"natb{b}",
                                            name=f"natb{b}")
                        nc.vector.memset(natb[0:1, 0:1], 0)
                        nc.gpsimd.dma_start(
                            natb[:].rearrange("p (t d) -> p t d", t=ST),
                            ctx16[row0:row0 + S, :].rearrange(
                                "(t p) d -> p t d", t=ST))
                        nat[b] = natb

                alpha16s = [None] * NG
                alphaTs = [None] * NG

                def smx_group(g):
                    """softmax for group g."""
                    r0 = g * GS
                    att_g = att_gs[g]
                    mask_g = smx.tile([GS, S], f32, tag="mask_g", name="mask_g")
                    nc.gpsimd.dma_start(mask_g[:], maskadd[r0:r0 + GS, :])
                    nc.vector.tensor_add(att_g[:], att_g[:], mask_g[:])
                    negmx = smx.tile([GS, 1], f32, tag="negmx", name="negmx")
                    nc.vector.tensor_reduce(negmx[:], att_g[:], axis=X, op=MAX, negate=True)
                    p16 = smx.tile([GS, S], fp16, tag="p16", name="p16")
                    nc.scalar.activation(p16[:], att_g[:], EXP, bias=negmx[:])
                    ssum = smx.tile([GS, 1], f32, tag="ssum", name="ssum")
                    nc.vector.tensor_reduce(ssum[:], p16[:], axis=X, op=ADD)
                    rsum = smx.tile([GS, 1], f32, tag="rsum", name="rsum")
                    nc.vector.reciprocal(rsum[:], ssum[:])
                    nc.vector.tensor_scalar_mul(att_g[:], p16[:], rsum[:])
                    nc.gpsimd.dma_start(alpha_out[r0:r0 + GS, :], att_g[:])
                    alpha16 = smx.tile([GS, S], fp16, tag="alpha16", name="alpha16",
                                       bufs=2)
                    nc.vector.tensor_scalar_mul(alpha16[:], p16[:], rsum[:])
                    alpha16s[g] = alpha16

                def passB_group(g):
                    """alphaT + weighted context sum for group g."""
                    r0 = g * GS
                    alpha16 = alpha16s[g]
                    c_all = p2w.tile([GS, D], fp16, tag="c_all", name="c_all")
                    c_alls[g] = c_all
                    # alphaT[st] [128, GS] via PE transpose
                    alphaT = [p2w.tile([128, GS], fp16, tag=f"alT{st}",
                                        name=f"alT{st}") for st in range(ST)]
                    alphaTs[g] = alphaT
                    for st in range(ST):
                        pt = mmps.tile([128, GS], fp16, tag="mm", name="pt16")
                        nc.tensor.transpose(pt[:], alpha16[:, st * 128:(st + 1) * 128],
                                            ident_sb[:, :])
                        nc.scalar.copy(alphaT[st][:], pt[:])
                    # c[b] = alpha[b] @ context[b]  (rows via DRAM bounce)
                    for j in range(GS):
                        b = r0 + j
                        pc = attps.tile([1, D], f32, tag="att", name="pc")
                        for st in range(ST):
                            nc.tensor.matmul(pc[:], alphaT[st][:, j:j + 1],
                                             nat[b][:, st * D:(st + 1) * D],
                                             start=(st == 0), stop=(st == ST - 1))
                        crow = p2w.tile([1, D], fp16, tag="crow", name="crow")
                        nc.vector.tensor_copy(crow[:], pc[0:1, :])
                        nc.gpsimd.dma_start(c_all[j:j + 1, :], crow[:])

                def proj_group(g):
                    """cT + output projection for group g."""
                    r0 = g * GS
                    c_all = c_alls[g]
                    cT = [p2w.tile([128, GS], fp16, tag=f"cT{k}", name=f"cT{k}")
                          for k in range(KT)]
                    for k in range(KT):
                        ptc = mmps.tile([128, GS], fp16, tag="mm", name="ptc")
                        nc.tensor.transpose(ptc[:], c_all[:, k * 128:(k + 1) * 128],
                                            ident_sb[:, :])
                        nc.scalar.copy(cT[k][:], ptc[:])
                    # hidden.T[:, group] = W_ctx @ c + b_ctx
                    hT = p2w.tile([128, MT * GS], f32, tag="hT", name="hT")
                    for m in range(MT):
                        ph = mmps.tile([128, GS], f32, tag="mm", name="ph")
                        for k in range(KT):
                            nc.tensor.matmul(ph[:], wctx[k][:, m * 128:(m + 1) * 128],
                                             cT[k][:], start=(k == 0), stop=False)
                        nc.tensor.matmul(ph[:], bctx_sb[:, m * 128:(m + 1) * 128],
                                         ones_sb[:, 0:GS], start=False, stop=True)
                        nc.scalar.copy(hT[:, m * GS:(m + 1) * GS], ph[:])
                    nc.gpsimd.dma_start(
                        hidT_out[:, r0:r0 + GS].rearrange("(m p) b -> p m b", m=MT),
                        hT[:].rearrange("p (m b) -> p m b", m=MT))

                for g in range(NG):
                    scores_group(g)
                for g in range(NG):
                    smx_group(g)
                for g in range(NG):
                    passB_group(g)
                for g in range(NG):
                    proj_group(g)

    nc.finalize()
    return nc


def _prep_core_inputs(inputs):
    """Host-side sharding + layout prep. Returns in_maps for 8 cores."""
    context = np.ascontiguousarray(inputs["context"], dtype=np.float32)
    inp = np.asarray(inputs["input"], dtype=np.float32)
    mask = np.asarray(inputs["mask"])
    W_in = np.asarray(inputs["W_in"], dtype=np.float32)
    b_in = np.asarray(inputs["b_in"], dtype=np.float32)
    W_ctx = np.asarray(inputs["W_ctx"], dtype=np.float32)
    b_ctx = np.asarray(inputs["b_ctx"], dtype=np.float32)
    V = np.asarray(inputs["V"], dtype=np.float32)

    ident16 = np.eye(GS, dtype=np.float16)
    rows16 = np.concatenate([
        (b_in + b_ctx).astype(np.float16), b_ctx.astype(np.float16),
        np.ones(BS, np.float16)]).reshape(1, 2 * H + BS)

    ctx16_c = []
    ctxT16_c = []
    for c in range(N_CORES):
        bsl = slice(c * BS, (c + 1) * BS)
        blk16 = context[bsl].astype(np.float16)
        ctx16_c.append(blk16.reshape(BS * S, D))
        ctxT16_c.append(np.ascontiguousarray(blk16.transpose(0, 2, 1)).reshape(BS * D, S))
    WpackA = np.empty((D, H + 1), np.float16)
    WpackA[:, 0:H] = W_ctx.T.astype(np.float16)
    WpackA[:, H] = V.astype(np.float16)
    WinT = W_in.T.astype(np.float16)
    in_maps = []
    for c in range(N_CORES):
        bsl = slice(c * BS, (c + 1) * BS)
        WpackB = np.empty((D, H + BS), np.float16)
        WpackB[:, 0:H] = WinT
        WpackB[:, H:] = inp[bsl].T.astype(np.float16)
        in_maps.append({
            "ctx16": ctx16_c[c],
            "ctxT16": ctxT16_c[c],
            "WpackA": WpackA,
            "WpackB": WpackB,
            "rows16": rows16,
            "maskadd": np.where(mask[bsl], np.float32(NEG), np.float32(0.0)),
            "ident16": ident16,
        })
    return in_maps


def kernel(**inputs):
    from concourse.bass_utils import run_bass_kernel_spmd

    if _cache["nc"] is None:
        _cache["nc"] = _build_nc()
    nc = _cache["nc"]

    in_maps = _prep_core_inputs(inputs)
    kwargs = {}
    trace = bool(os.environ.get("KERNEL_TRACE"))
    if trace:
        try:
            import prof_hook
            prof_hook.install()
        except Exception:
            pass
        kwargs = {"trace": True, "tmpdir": os.environ.get("KERNEL_TRACE_DIR") or None}
    res = run_bass_kernel_spmd(nc, in_maps, core_ids=list(range(N_CORES)), **kwargs)
    _cache["last_exec_ns"] = res.exec_time_ns

    hidden = np.empty((B, H), np.float32)
    alpha = np.empty((B, S), np.float32)
    for c in range(N_CORES):
        bsl = slice(c * BS, (c + 1) * BS)
        alpha[bsl] = res.results[c]["alpha_out"]
        hidden[bsl] = res.results[c]["hidT_out"].T
    return hidden, alpha


# revision 27
# speedup vs baseline: 1.0191x; 1.0106x over previous
"""Bass/Trainium2 kernel for nn_Attention (Bahdanau-style additive attention).

reference:
    inp = input @ W_in.T + b_in                                  # [B, H]
    ctx = einsum('bsd,hd->bhs', context, W_ctx) + b_ctx          # [B, H, S]
    att = einsum('h,bhs->bs', V, tanh(inp[:,:,None] + ctx))      # [B, S]
    att = where(mask, -inf, att); alpha = softmax(att, -1)       # [B, S]
    hidden = einsum('bhs,bs->bh', ctx, alpha)                    # [B, H]

Key restructuring: hidden = W_ctx @ (alpha @ context) + b_ctx (since sum(alpha)=1),
so the big [B,H,S] ctx tensor is only ever materialized tile-by-tile in PSUM.

Sharding: data-parallel over batch B across 8 cores (16 batches/core).
Compute dtype fp16 (alpha abs err ~2e-3 vs f32 reference), f32 PSUM accumulate.
Batches processed in 2 groups of 8 so group-0 softmax/pass-B overlaps group-1
score compute.
"""
import os
import numpy as np

B, S, D, H = 128, 1024, 512, 512
N_CORES = 8
BS = B // N_CORES   # batches per core
NG = 2              # batch groups per core
GS = BS // NG       # batches per group
KT = D // 128       # 4 contraction tiles
MT = H // 128       # 4 output tiles
ST = S // 128       # 8 sequence tiles
NEG = -1.0e30

_cache = {"nc": None}


def _build_nc():
    import concourse.bacc as bacc
    import concourse.tile as tile
    from concourse import mybir

    fp16 = mybir.dt.float16
    f32 = mybir.dt.float32
    TANH = mybir.ActivationFunctionType.Tanh
    EXP = mybir.ActivationFunctionType.Exp
    X = mybir.AxisListType.X
    MAX = mybir.AluOpType.max
    ADD = mybir.AluOpType.add

    nc = bacc.Bacc("TRN2", target_bir_lowering=False, debug=False,
                   num_devices=N_CORES)

    ctx16 = nc.declare_dram_parameter("ctx16", [BS * S, D], fp16, isOutput=False)
    ctxT16 = nc.declare_dram_parameter("ctxT16", [BS * D, S], fp16, isOutput=False)
    WCA = H + 1              # [W_ctxT | V]
    WCB = H + BS             # [W_inT | inputT]
    WpackA = nc.declare_dram_parameter("WpackA", [D, WCA], fp16, isOutput=False)
    WpackB = nc.declare_dram_parameter("WpackB", [D, WCB], fp16, isOutput=False)
    rows16 = nc.declare_dram_parameter("rows16", [1, 2 * H + BS], fp16, isOutput=False)
    maskadd = nc.declare_dram_parameter("maskadd", [BS, S], f32, isOutput=False)
    ident16 = nc.declare_dram_parameter("ident16", [GS, GS], fp16, isOutput=False)

    alpha_out = nc.declare_dram_parameter("alpha_out", [BS, S], f32, isOutput=True)
    hidT_out = nc.declare_dram_parameter("hidT_out", [H, BS], f32, isOutput=True)

    with tile.TileContext(nc) as tc:
        with tc.tile_pool(name="const", bufs=1) as cpool, \
             tc.tile_pool(name="natc", bufs=1) as natpool, \
             tc.tile_pool(name="dramp", bufs=1, space="DRAM") as dpool:

            # weights / small constants (live whole kernel)
            wpa = cpool.tile([128, KT * WCA], fp16, tag="wpa", name="wpa")
            nc.sync.dma_start(
                wpa[:].rearrange("p (k c) -> p k c", k=KT),
                WpackA[:, :].rearrange("(k p) c -> p k c", k=KT))
            wctx = [wpa[:, k * WCA:k * WCA + H] for k in range(KT)]
            vcol = [wpa[:, k * WCA + H:k * WCA + WCA] for k in range(KT)]
            rp = cpool.tile([1, 2 * H + BS], fp16, tag="rp", name="rp")
            wpb = cpool.tile([128, KT * WCB], fp16, tag="wpb", name="wpb")
            winT = [wpb[:, k * WCB:k * WCB + H] for k in range(KT)]
            inT = [wpb[:, k * WCB + H:k * WCB + WCB] for k in range(KT)]
            brow_sb = rp[:, 0:H]
            bctx_sb = rp[:, H:2 * H]
            ones_sb = rp[:, 2 * H:2 * H + BS]
            ident_sb = cpool.tile([GS, GS], fp16, tag="ident", name="ident")
            nc.gpsimd.dma_start(ident_sb[:], ident16[:, :])
            # per-(h,b) tanh bias: W_in @ input.T + (b_in + b_ctx), f32, [128, BS] x MT
            ibias = [cpool.tile([128, BS], f32, tag=f"ibias{m}", name=f"ibias{m}") for m in range(MT)]

            nat = [None] * BS  # natural-layout ctx tiles for pass B
            att_gs = [None] * NG
            c_alls = [None] * NG

            with tc.tile_pool(name="ctxT", bufs=3) as tpool, \
                 tc.tile_pool(name="tanh", bufs=2) as hpool, \
                 tc.tile_pool(name="mmps", bufs=6, space="PSUM") as mmps, \
                 tc.tile_pool(name="attps", bufs=2, space="PSUM") as attps, \
                 tc.tile_pool(name="smx", bufs=1) as smx, \
                 tc.tile_pool(name="p2w", bufs=2) as p2w:

                def phase0():
                    # load rows + W_inT|inputT; ibias = W_in@input.T + biases
                    nc.sync.dma_start(rp[:], rows16[:, :])
                    nc.sync.dma_start(
                        wpb[:].rearrange("p (k c) -> p k c", k=KT),
                        WpackB[:, :].rearrange("(k p) c -> p k c", k=KT))
                    for m in range(MT):
                        ps = attps.tile([128, BS], f32, tag="att", name="p0ps")
                        for k in range(KT):
                            nc.tensor.matmul(ps[:], winT[k][:, m * 128:(m + 1) * 128],
                                             inT[k][:], start=(k == 0), stop=False)
                        nc.tensor.matmul(ps[:], brow_sb[:, m * 128:(m + 1) * 128],
                                         ones_sb[:], start=False, stop=True)
                        nc.scalar.copy(ibias[m][:], ps[:])

                def scores_group(g):
                    """Phase 1 for batches g*GS .. (g+1)*GS-1: att rows -> att_g."""
                    att_g = smx.tile([GS, S], f32, tag="att_g", name="att_g", bufs=2)
                    att_gs[g] = att_g
                    for j in range(GS):
                        b = g * GS + j
                        row0 = b * S
                        trow0 = b * D
                        # pre-transposed context: one 1MB DMA at steady state;
                        # k-split for the first batches so the PE can chase
                        # per-k arrivals during DMA ramp-up
                        ctxTb = tpool.tile([128, KT * S], fp16, tag="ctxTb",
                                           name="ctxTb")
                        if b < 2:
                            for k in range(KT):
                                nc.sync.dma_start(
                                    ctxTb[:, k * S:(k + 1) * S],
                                    ctxT16[trow0 + k * 128:trow0 + (k + 1) * 128, :])
                        else:
                            nc.sync.dma_start(
                                ctxTb[:].rearrange("p (k s) -> p k s", k=KT),
                                ctxT16[trow0:trow0 + D, :].rearrange(
                                    "(k p) s -> p k s", k=KT))
                        tanh_sb = [hpool.tile([128, S], fp16, tag=f"tanh{m}",
                                              name=f"tanh{m}") for m in range(MT)]
                        chunk_ps = {}
                        for n in range(2):
                            for m in range(MT):
                                ps = mmps.tile([128, 512], f32, tag="mm", name="mm")
                                for k in range(KT):
                                    nc.tensor.matmul(
                                        ps[:], wctx[k][:, m * 128:(m + 1) * 128],
                                        ctxTb[:, k * S + n * 512:k * S + (n + 1) * 512],
                                        start=(k == 0), stop=(k == KT - 1))
                                if b == 0 and n == 0:
                                    chunk_ps[m] = ps
                                    continue
                                nc.scalar.activation(
                                    tanh_sb[m][:, n * 512:(n + 1) * 512], ps[:],
                                    TANH, bias=ibias[m][:, b:b + 1])
                            if b == 0 and n == 0:
                                phase0()
                                for m in range(MT):
                                    nc.scalar.activation(
                                        tanh_sb[m][:, 0:512], chunk_ps[m][:],
                                        TANH, bias=ibias[m][:, b:b + 1])
                        for n in range(2):
                            ap = attps.tile([1, 512], f32, tag="att", name="att")
                            for m in range(MT):
                                nc.tensor.matmul(ap[:], vcol[m][:, 0:1],
                                                 tanh_sb[m][:, n * 512:(n + 1) * 512],
                                                 start=(m == 0), stop=(m == MT - 1))
                            arow = hpool.tile([1, 512], f32, tag="arow", name="arow")
                            nc.vector.tensor_copy(arow[:], ap[0:1, :])
                            nc.gpsimd.dma_start(
                                att_g[j:j + 1, n * 512:(n + 1) * 512], arow[:])
                        # natural load for pass B (SWDGE), one 1MB DMA - after
                        # compute so the score-path ctxT DMAs get HBM priority
                        natb = natpool.tile([128, ST * D], fp16, tag=f"natb{b}",
                                            name=f"natb{b}")
                        nc.vector.memset(natb[0:1, 0:1], 0)
                        nc.gpsimd.dma_start(
                            natb[:].rearrange("p (t d) -> p t d", t=ST),
                            ctx16[row0:row0 + S, :].rearrange(
                                "(t p) d -> p t d", t=ST))
                        nat[b] = natb

                alpha16s = [None] * NG
                alphaTs = [None] * NG

                def smx_group(g):
                    """softmax for group g."""
                    r0 = g * GS
                    att_g = att_gs[g]
                    mask_g = smx.tile([GS, S], f32, tag="mask_g", name="mask_g")
                    nc.gpsimd.dma_start(mask_g[:], maskadd[r0:r0 + GS, :])
                    nc.vector.tensor_add(att_g[:], att_g[:], mask_g[:])
                    negmx = smx.tile([GS, 1], f32, tag="negmx", name="negmx")
                    nc.vector.tensor_reduce(negmx[:], att_g[:], axis=X, op=MAX, negate=True)
                    p16 = smx.tile([GS, S], fp16, tag="p16", name="p16")
                    nc.scalar.activation(p16[:], att_g[:], EXP, bias=negmx[:])
                    ssum = smx.tile([GS, 1], f32, tag="ssum", name="ssum")
                    nc.vector.tensor_reduce(ssum[:], p16[:], axis=X, op=ADD)
                    rsum = smx.tile([GS, 1], f32, tag="rsum", name="rsum")
                    nc.vector.reciprocal(rsum[:], ssum[:])
                    nc.vector.tensor_scalar_mul(att_g[:], p16[:], rsum[:])
                    nc.gpsimd.dma_start(alpha_out[r0:r0 + GS, :], att_g[:])
                    alpha16 = smx.tile([GS, S], fp16, tag="alpha16", name="alpha16",
                                       bufs=2)
                    nc.vector.tensor_scalar_mul(alpha16[:], p16[:], rsum[:])
                    alpha16s[g] = alpha16

                def passB_group(g):
                    """alphaT + weighted context sum for group g."""
                    r0 = g * GS
                    alpha16 = alpha16s[g]
                    c_all = p2w.tile([GS, D], fp16, tag="c_all", name="c_all")
                    c_alls[g] = c_all
                    # alphaT[st] [128, GS] via PE transpose
                    alphaT = [p2w.tile([128, GS], fp16, tag=f"alT{st}",
                                        name=f"alT{st}") for st in range(ST)]
                    alphaTs[g] = alphaT
                    for st in range(ST):
                        pt = mmps.tile([128, GS], fp16, tag="mm", name="pt16")
                        nc.tensor.transpose(pt[:], alpha16[:, st * 128:(st + 1) * 128],
                                            ident_sb[:, :])
                        nc.scalar.copy(alphaT[st][:], pt[:])
                    # c[b] = alpha[b] @ context[b]  (rows via DRAM bounce)
                    for j in range(GS):
                        b = r0 + j
                        pc = attps.tile([1, D], f32, tag="att", name="pc")
                        for st in range(ST):
                            nc.tensor.matmul(pc[:], alphaT[st][:, j:j + 1],
                                             nat[b][:, st * D:(st + 1) * D],
                                             start=(st == 0), stop=(st == ST - 1))
                        crow = p2w.tile([1, D], fp16, tag="crow", name="crow")
                        nc.vector.tensor_copy(crow[:], pc[0:1, :])
                        nc.gpsimd.dma_start(c_all[j:j + 1, :], crow[:])

                def proj_group(g):
                    """cT + output projection for group g."""
                    r0 = g * GS
                    c_all = c_alls[g]
                    cT = [p2w.tile([128, GS], fp16, tag=f"cT{k}", name=f"cT{k}")
                          for k in range(KT)]
                    for k in range(KT):
                        ptc = mmps.tile([128, GS], fp16, tag="mm", name="ptc")
                        nc.tensor.transpose(ptc[:], c_all[:, k * 128:(k + 1) * 128],
                                            ident_sb[:, :])
                        nc.scalar.copy(cT[k][:], ptc[:])
                    # hidden.T[:, group] = W_ctx @ c + b_ctx
                    hT = p2w.tile([128, MT * GS], f32, tag="hT", name="hT")
                    for m in range(MT):
                        ph = mmps.tile([128, GS], f32, tag="mm", name="ph")
                        for k in range(KT):
                            nc.tensor.matmul(ph[:], wctx[k][:, m * 128:(m + 1) * 128],
                                             cT[k][:], start=(k == 0), stop=False)
                        nc.tensor.matmul(ph[:], bctx_sb[:, m * 128:(m + 1) * 128],
                                         ones_sb[:, 0:GS], start=False, stop=True)
                        nc.scalar.copy(hT[:, m * GS:(m + 1) * GS], ph[:])
                    nc.gpsimd.dma_start(
                        hidT_out[:, r0:r0 + GS].rearrange("(m p) b -> p m b", m=MT),
                        hT[:].rearrange("p (m b) -> p m b", m=MT))

                for g in range(NG):
                    scores_group(g)
                for g in range(NG):
                    smx_group(g)
                for g in range(NG):
                    passB_group(g)
                for g in range(NG):
                    proj_group(g)

    nc.finalize()
    return nc


def _prep_core_inputs(inputs):
    """Host-side sharding + layout prep. Returns in_maps for 8 cores."""
    context = np.ascontiguousarray(inputs["context"], dtype=np.float32)
    inp = np.asarray(inputs["input"], dtype=np.float32)
    mask = np.asarray(inputs["mask"])
    W_in = np.asarray(inputs["W_in"], dtype=np.float32)
    b_in = np.asarray(inputs["b_in"], dtype=np.float32)
    W_ctx = np.asarray(inputs["W_ctx"], dtype=np.float32)
    b_ctx = np.asarray(inputs["b_ctx"], dtype=np.float32)
    V = np.asarray(inputs["V"], dtype=np.float32)

    ident16 = np.eye(GS, dtype=np.float16)
    rows16 = np.concatenate([
        (b_in + b_ctx).astype(np.float16), b_ctx.astype(np.float16),
        np.ones(BS, np.float16)]).reshape(1, 2 * H + BS)

    ctx16_c = []
    ctxT16_c = []
    for c in range(N_CORES):
        bsl = slice(c * BS, (c + 1) * BS)
        blk16 = context[bsl].astype(np.float16)
        ctx16_c.append(blk16.reshape(BS * S, D))
        ctxT16_c.append(np.ascontiguousarray(blk16.transpose(0, 2, 1)).reshape(BS * D, S))
    WpackA = np.empty((D, H + 1), np.float16)
    WpackA[:, 0:H] = W_ctx.T.astype(np.float16)
    WpackA[:, H] = V.astype(np.float16)
    WinT = W_in.T.astype(np.float16)
    in_maps = []
    for c in range(N_CORES):
        bsl = slice(c * BS, (c + 1) * BS)
        WpackB = np.empty((D, H + BS), np.float16)
        WpackB[:, 0:H] = WinT
        WpackB[:, H:] = inp[bsl].T.astype(np.float16)
        in_maps.append({
            "ctx16": ctx16_c[c],
            "ctxT16": ctxT16_c[c],
            "WpackA": WpackA,
            "WpackB": WpackB,
            "rows16": rows16,
            "maskadd": np.where(mask[bsl], np.float32(NEG), np.float32(0.0)),
            "ident16": ident16,
        })
    return in_maps


def kernel(**inputs):
    from concourse.bass_utils import run_bass_kernel_spmd

    if _cache["nc"] is None:
        _cache["nc"] = _build_nc()
    nc = _cache["nc"]

    in_maps = _prep_core_inputs(inputs)
    kwargs = {}
    trace = bool(os.environ.get("KERNEL_TRACE"))
    if trace:
        try:
            import prof_hook
            prof_hook.install()
        except Exception:
            pass
        kwargs = {"trace": True, "tmpdir": os.environ.get("KERNEL_TRACE_DIR") or None}
    res = run_bass_kernel_spmd(nc, in_maps, core_ids=list(range(N_CORES)), **kwargs)
    _cache["last_exec_ns"] = res.exec_time_ns

    hidden = np.empty((B, H), np.float32)
    alpha = np.empty((B, S), np.float32)
    for c in range(N_CORES):
        bsl = slice(c * BS, (c + 1) * BS)
        alpha[bsl] = res.results[c]["alpha_out"]
        hidden[bsl] = res.results[c]["hidT_out"].T
    return hidden, alpha
